# revision 36
# baseline (speedup 1.0000x reference)
"""Ragged segment mean kernel for Trainium2 (8 NeuronCores, data-parallel).

Problem: seq [64, 2048, 1024] f32, begin/end [64] i32/i64.
Output: out[i] = mean(seq[i, begin[i]:end[i], :])  -> [64, 1024] f32.

Strategy: pure data parallel over the batch, 8 samples per core, with a
host-directed "slot" architecture so each core reads only the rows its
segments actually cover:

- A slot is a 256-row (1 MiB) contiguous read of the core's seq shard
  at a runtime row offset (register-loaded from a small int32 input).
  Sample i's segment [begin, end) is covered by ~ceil(span/256) slots
  starting at begin + 256k, clamped to stay inside the sample; the
  mask bounds keep coverage exact even when clamping makes slots
  overlap, so any 0 <= begin < end <= L is handled.
- The host bin-packs samples onto cores (a permutation of the batch)
  to equalize per-core slot counts, builds each slot's 0/1 row mask
  (segment membership), and pads cores to a common slot count S with
  zero-mask slots. The last 256-row position is split into two
  128-row positions to halve the end-of-kernel drain chain. S is
  input-dependent; compiled kernels are cached per S, so unusual
  inputs at worst trigger a recompile, never a wrong result.

Per slot the masked row-sum runs on the PE: for each 128-row chunk,
acc[8, 512] += mask[128, 8].T @ chunk[128, 512], accumulated in PSUM
over all slots (the mask column routes rows to the right output row),
then scaled by 1/count and stored.

fp32 matmuls stream at 4 cycles/row on the PE (2 half-speed passes),
which would make the PE the bottleneck. Instead the data is split
exactly as x = hi + resid with hi = round_f32r(x) (ACT engine) and
resid = x - hi (DVE; exactly representable), and two float32r matmul
pairs (1 cycle/row) accumulate hi and resid into the same PSUM banks.
The result is exact to fp32 accumulation noise (~1e-7 rel).
"""

import numpy as np

import concourse.bacc as bacc
import concourse.bass as bass
import concourse.mybir as mybir
import concourse.tile as tile
from concourse.bass_utils import run_bass_kernel_spmd

B, L, D = 64, 2048, 1024
NCORES = 8
BP = B // NCORES              # 8 samples per core
NROW = BP * L                 # 16384 rows per core
U_ROWS = 256                  # rows per slot -> 1 MiB per dma_start
JPG = U_ROWS // 128           # 2 chunks of 128 rows per slot
FREE = 512                    # PSUM bank limit for matmul N
NMM = D // FREE               # 2 matmuls per chunk
S_BUCKET = 1                  # round slot count up to a multiple of this

_nc_cache = {}


def _build_nc(S):
    nc = bacc.Bacc("TRN2", target_bir_lowering=False)
    seq = nc.dram_tensor("seq", [NROW, D], mybir.dt.float32, kind="ExternalInput")
    NCH_ = (S - 1) * JPG + 2
    maskt = nc.dram_tensor(
        "maskt", [128, NCH_ * BP], mybir.dt.float32, kind="ExternalInput"
    )
    invc = nc.dram_tensor("invc", [BP, 1], mybir.dt.float32, kind="ExternalInput")
    beg = nc.dram_tensor("beg", [S + 1, 1], mybir.dt.int32, kind="ExternalInput")
    out = nc.dram_tensor("out", [BP, D], mybir.dt.float32, kind="ExternalOutput")

    f32 = mybir.dt.float32
    f32r = mybir.dt.float32r
    GF = U_ROWS * D // 128  # free size of one slot tile (2048)
    # Positions: S-1 full slots then 2 half slots (the tail taper halves
    # the end-of-kernel ACT->DVE->PE drain chain). Host supplies S+1
    # offsets and masks laid out accordingly.
    POS = [(s, U_ROWS) for s in range(S - 1)] + [(S - 1, 128), (S, 128)]
    NCH = (S - 1) * JPG + 2  # total 128-row chunks across positions

    with tile.TileContext(nc) as tc:
        with (
            tc.tile_pool(name="const", bufs=1) as cpool,
            tc.tile_pool(name="seqp", bufs=6) as spool,
            tc.tile_pool(name="accp", bufs=1, space="PSUM") as ppool,
            tc.tile_pool(name="resp", bufs=1) as rpool,
        ):
            # bg first: the slot DMAs depend only on it. The mask/iv DMAs
            # are emitted after the first slot DMA so slot 0 issues ASAP.
            bg = cpool.tile([S + 1, 1], mybir.dt.int32, tag="bg")
            nc.sync.dma_start(out=bg[:], in_=beg[:])
            mt = cpool.tile([128, NCH * BP], f32)
            mr = cpool.tile([128, NCH * BP], f32r, tag="mr")
            iv = cpool.tile([BP, 1], f32)
            iv2 = cpool.tile([BP, 1], f32, tag="iv2")
            warm = ppool.tile([BP, BP], f32, tag="warm")

            acc = ppool.tile([BP, D], f32)
            ch = 0  # global 128-row chunk counter (mask column base)
            for pi, (s, rows) in enumerate(POS):
                jpg = rows // 128
                gf = rows * D // 128
                r = nc.alloc_register(mybir.EngineType.SP, f"rs{pi}")
                nc.sync.reg_load(r, bg[pi : pi + 1, 0:1])
                off = nc.snap(r, min_val=0, max_val=NROW - rows)
                t = spool.tile([128, GF], f32)
                src = seq[bass.ds(off, rows), :].rearrange(
                    "(p j) d -> p (j d)", p=128
                )
                nc.sync.dma_start(out=t[:, 0:gf], in_=src)
                if pi == 0:
                    # constants: mask DMA + f32r round (exact for 0/1),
                    # inv-count DMA + DVE pre-touch, and a warmup matmul
                    # consuming only the mask tile -- the self-loading f32r
                    # LDWEIGHTS encoding fits a single sync wait, so the
                    # first real matmul must not wait on both the mask copy
                    # and the seq pipeline; this absorbs the mask dep into
                    # the PE clock.
                    nc.sync.dma_start(out=mt[:], in_=maskt[:])
                    nc.vector.tensor_copy(out=mr[:], in_=mt[:])
                    nc.sync.dma_start(out=iv[:], in_=invc[:])
                    nc.vector.tensor_copy(out=iv2[:], in_=iv[:])
                    nc.tensor.matmul(
                        out=warm[:],
                        lhsT=mr[:, 0:BP],
                        rhs=mr[:, 0:BP],
                        start=True,
                        stop=True,
                    )
                # exact split x = hi + resid in f32r precision
                hi = spool.tile([128, GF], f32r, tag="hi")
                nc.scalar.copy(out=hi[:, 0:gf], in_=t[:, 0:gf])
                rs = spool.tile([128, GF], f32r, tag="rs")
                nc.vector.tensor_tensor(
                    out=rs[:, 0:gf],
                    in0=t[:, 0:gf],
                    in1=hi[:, 0:gf].bitcast(f32),
                    op=mybir.AluOpType.subtract,
                )
                for j in range(jpg):
                    lhs = mr[:, (ch + j) * BP : (ch + j + 1) * BP]
                    for h in range(NMM):
                        sl = slice(j * D + h * FREE, j * D + (h + 1) * FREE)
                        nc.tensor.matmul(
                            out=acc[:, h * FREE : (h + 1) * FREE],
                            lhsT=lhs,
                            rhs=hi[:, sl],
                            start=(pi == 0 and j == 0),
                            stop=False,
                        )
                        nc.tensor.matmul(
                            out=acc[:, h * FREE : (h + 1) * FREE],
                            lhsT=lhs,
                            rhs=rs[:, sl],
                            start=False,
                            stop=(pi == len(POS) - 1 and j == jpg - 1),
                        )
                ch += jpg

            res = rpool.tile([BP, D], f32)
            nc.vector.tensor_scalar_mul(out=res[:], in0=acc[:], scalar1=iv2[:])
            nc.sync.dma_start(out=out[:], in_=res[:])
    nc.compile()
    return nc


def _sample_units(b, e):
    """Slots covering [b, e) of one sample: (local row start, lo, hi).

    Slot k starts at b + 256k, clamped to stay inside [0, L); the mask
    bounds [lo, hi) cover each segment row exactly once even when the
    clamp makes slots overlap. Works for any 0 <= b < e <= L.
    """
    units = []
    cov = b
    k = 0
    while cov < e:
        s0 = min(b + k * U_ROWS, L - U_ROWS)
        hi = min(e, s0 + U_ROWS)
        units.append((s0, cov, hi))
        cov = hi
        k += 1
    return units


def _plan(begin, end):
    """Bin-pack samples onto cores; return (perm, S).

    perm[ci*BP + i_local] = original sample index.
    """
    units = np.array(
        [len(_sample_units(int(b), int(e))) for b, e in zip(begin, end)],
        dtype=np.int64,
    )
    order = np.argsort(-units, kind="stable")
    loads = [0] * NCORES
    members = [[] for _ in range(NCORES)]
    for si in order:
        ci = loads.index(min(loads))
        if len(members[ci]) >= BP:
            # this core is full; pick the least-loaded core with room
            ci = min(
                (c for c in range(NCORES) if len(members[c]) < BP),
                key=lambda c: loads[c],
            )
        loads[ci] += int(units[si])
        members[ci].append(int(si))
    # pad cores to exactly BP samples (all samples used exactly once)
    perm = np.array([si for ci in range(NCORES) for si in members[ci]], dtype=np.int64)
    assert len(perm) == B and len(set(perm.tolist())) == B
    S = max(loads)
    S = -(-S // S_BUCKET) * S_BUCKET
    return perm, S


def _make_in_maps(seq, begin, end, perm, S):
    NCH = (S - 1) * JPG + 2
    in_maps = []
    p = np.arange(128)
    for ci in range(NCORES):
        samples = perm[ci * BP : (ci + 1) * BP]
        b = begin[samples].astype(np.int64)
        e = end[samples].astype(np.int64)
        span = e - b
        units = []  # (core row offset, local sample, mask lo, mask hi, s0)
        for i in range(BP):
            for s0, lo, hi in _sample_units(int(b[i]), int(e[i])):
                units.append((i * L + s0, i, lo, hi, s0))
        assert len(units) <= S, (len(units), S)
        offs = np.zeros((S + 1, 1), dtype=np.int32)
        mt = np.zeros((128, NCH * BP), dtype=np.float32)
        fulls = units[: S - 1]
        tail = units[S - 1] if len(units) == S else None
        for pi, (off, i, lo, hi, s0) in enumerate(fulls):
            offs[pi, 0] = off
            for j in range(JPG):
                # full-slot tile[p, j*D+d] holds slot row JPG*p + j
                w = s0 + JPG * p + j
                mt[:, (pi * JPG + j) * BP + i] = ((w >= lo) & (w < hi)).astype(
                    np.float32
                )
        if tail is not None:
            off, i, lo, hi, s0 = tail
            for h2 in range(2):
                offs[S - 1 + h2, 0] = off + 128 * h2
                w = s0 + 128 * h2 + p  # half-slot tile[p, d] holds row p
                ch = (S - 1) * JPG + h2
                mt[:, ch * BP + i] = ((w >= lo) & (w < hi)).astype(np.float32)
        inv = (1.0 / span.astype(np.float64)).astype(np.float32).reshape(BP, 1)
        in_maps.append(
            {
                "seq": np.ascontiguousarray(
                    seq[samples].reshape(NROW, D), dtype=np.float32
                ),
                "maskt": mt,
                "invc": inv,
                "beg": offs,
            }
        )
    return in_maps


def _axon_reset():
    """Best-effort NeuronCore reset (recovers a device wedged by an
    earlier failed run in the same container)."""
    try:
        import ctypes

        import jax

        jax.devices()
        lib = ctypes.CDLL("/opt/axon/libaxon_pjrt.so")
        lib.axon_reset.restype = ctypes.c_int64
        lib.axon_reset()
    except Exception:
        pass


def _run(seq, begin, end, trace=False):
    seq = np.asarray(seq)
    begin = np.asarray(begin).astype(np.int64)
    end = np.asarray(end).astype(np.int64)
    perm, S = _plan(begin, end)
    if S not in _nc_cache:
        _nc_cache[S] = _build_nc(S)
    in_maps = _make_in_maps(seq, begin, end, perm, S)
    try:
        res = run_bass_kernel_spmd(
            _nc_cache[S], in_maps, list(range(NCORES)), trace=trace
        )
    except Exception:
        _axon_reset()
        res = run_bass_kernel_spmd(
            _nc_cache[S], in_maps, list(range(NCORES)), trace=trace
        )
    permuted = np.concatenate(
        [res.results[ci]["out"] for ci in range(NCORES)], axis=0
    )
    out = np.empty_like(permuted)
    out[perm] = permuted
    return out, res


def kernel(seq, begin, end):
    out, _ = _run(seq, begin, end, trace=False)
    return out
